# revision 7
# baseline (speedup 1.0000x reference)
"""AttnBlock (GroupNorm + 8-head self-attention + proj + residual) on 8 trn2 cores.

Sharding: one attention head per core, both batch elements on every core.
Each core computes its head's contribution to the output projection
(o_head @ Wo[:, head].T) as a full-shape partial; the host sums the 8
partials, adds bo and the residual x.

Per-core layouts (partition dim first):
  h.T      [C=512 (4 tiles of 128), N=4096] bf16   channels-first, as x arrives
  q.T/k.T  [128, 4096] bf16   rows 0:64 = batch0 head, rows 64:128 = batch1 head
  v_ext    [128 j-tile, 32, 65] bf16 per batch; col 64 = 1.0 (softmax denom trick)
  S.T      psum [128 j, 1024] f32: cols 0:512 batch0, 512:1024 batch1
  P.T      exp(S.T * 1/8) bf16 (no max subtraction: |S| < ~8 for this data)
  o.T      psum [65, 512] per batch: rows 0:64 = unnormalized o.T, row 64 = denom
"""

import numpy as np

NUM_HEADS = 8
B, C, H, W = 2, 512, 64, 64
N = H * W            # 4096
HD = C // NUM_HEADS  # 64
GROUPS = 32
EPS = 1e-5
NIC = 8              # i-chunks of 512
NJT = 32             # j-tiles of 128
CT = 4               # channel tiles of 128
SM_SCALE = 1.0 / 8.0  # 1/sqrt(HD)

_CACHE = {}


def _make_split_drain_tc(tile_mod, nc):
    """TileContext whose final drain splits its semaphore waits across
    nop instructions (this walrus build rejects >2 waits on one Drain)."""
    from concourse.tile import ScopedClock
    from concourse.tile_sem_assignment import VectorClock

    class SplitDrainTC(tile_mod.TileContext):
        def _drain_and_barrier(self, tick_clock, wait_clock):
            vec = list(
                eval(repr(tick_clock.global_clock).replace("VectorClock(", "").rstrip(")"))
            )
            for i, v in enumerate(vec):
                if v > 0:
                    partial = [v if j == i else 0 for j in range(len(vec))]
                    nop = self.nc.sync.nop()
                    wait_clock.add_sem_waits(
                        nop.ins, ScopedClock({None: VectorClock(partial)})
                    )
            self.nc.sync.drain()
            self.nc.all_engine_barrier()
            popped = self.nc._tile_sem_poison_stack.pop()
            assert popped is self._sem_poison
            self.nc.clear_and_free_semaphores(list(self.sems.allocated().values()))
            self.nc.all_engine_barrier()

    return SplitDrainTC(nc)


def _split_excess_waits(nc, mybir, limit=1):
    """This walrus build rejects >1 sync wait on one instruction; hoist the
    excess onto single-wait NoOps inserted just before, on the same engine."""
    fn = nc.m.functions[0]
    ctr = 0
    for bb in fn.blocks:
        new_insts = []
        changed = False
        for inst in bb.instructions:
            si = inst.sync_info
            if si is not None and si.on_wait and len(si.on_wait) > limit:
                waits = list(si.on_wait)
                excess, keep = waits[:-limit], waits[-limit:]
                for w in excess:
                    nop = mybir.InstNoOp(
                        name=f"waitsplit_{ctr}",
                        engine=inst.engine,
                        sync_info=mybir.SyncInfo(on_wait=[w], on_update=[]),
                    )
                    ctr += 1
                    new_insts.append(nop)
                inst.sync_info = mybir.SyncInfo(
                    on_wait=keep, on_update=list(si.on_update)
                )
                changed = True
            new_insts.append(inst)
        if changed:
            try:
                bb.instructions[:] = new_insts
            except TypeError:
                bb.instructions = new_insts


def build_program():
    import concourse.bass as bass
    import concourse.tile as tile
    from concourse import mybir

    f32 = mybir.dt.float32
    bf16 = mybir.dt.bfloat16
    mult = mybir.AluOpType.mult
    add = mybir.AluOpType.add
    subtract = mybir.AluOpType.subtract
    AF = mybir.ActivationFunctionType

    nc = bass.Bass("TRN2", debug=False, num_devices=NUM_HEADS)

    xbf = nc.declare_dram_parameter("xbf", [B, C, N], bf16, isOutput=False)
    wq_t = nc.declare_dram_parameter("wq_t", [C, HD], bf16, isOutput=False)
    wk_t = nc.declare_dram_parameter("wk_t", [C, HD], bf16, isOutput=False)
    wv_t = nc.declare_dram_parameter("wv_t", [C, HD], bf16, isOutput=False)
    wo_t = nc.declare_dram_parameter("wo_t", [HD, C], bf16, isOutput=False)
    bqk2 = nc.declare_dram_parameter("bqk2", [128, 2], f32, isOutput=False)  # col0 bq dup, col1 bk dup
    bv_p = nc.declare_dram_parameter("bv", [HD], f32, isOutput=False)
    gam = nc.declare_dram_parameter("gam", [C, 1], f32, isOutput=False)
    bet = nc.declare_dram_parameter("bet", [C, 1], f32, isOutput=False)
    ind16 = nc.declare_dram_parameter("ind16", [128, 8], f32, isOutput=False)
    exp8 = nc.declare_dram_parameter("exp8", [8, 128], f32, isOutput=False)
    out = nc.declare_dram_parameter("out", [B, C, N], f32, isOutput=True)

    tc = _make_split_drain_tc(tile, nc)
    with tc:
        from contextlib import ExitStack

        with ExitStack() as ctx:
            consts = ctx.enter_context(tc.tile_pool(name="consts", bufs=1))
            xpool = ctx.enter_context(tc.tile_pool(name="xpool", bufs=8))
            hpool = ctx.enter_context(tc.tile_pool(name="hpool", bufs=8))
            qkpool = ctx.enter_context(tc.tile_pool(name="qkpool", bufs=2))
            vpool = ctx.enter_context(tc.tile_pool(name="vpool", bufs=2))
            gnsb = ctx.enter_context(tc.tile_pool(name="gnsb", bufs=4))
            small = ctx.enter_context(tc.tile_pool(name="small", bufs=4))
            ptpool = ctx.enter_context(tc.tile_pool(name="ptpool", bufs=3))
            onpool = ctx.enter_context(tc.tile_pool(name="onpool", bufs=2))
            outp = ctx.enter_context(tc.tile_pool(name="outp", bufs=3))

            # ---------- constants ----------
            wq_sb = consts.tile([128, CT, HD], bf16)
            wk_sb = consts.tile([128, CT, HD], bf16)
            wv_sb = consts.tile([128, CT, HD], bf16)
            for wsb, wdr in ((wq_sb, wq_t), (wk_sb, wk_t), (wv_sb, wv_t)):
                # DRAM [C, HD] row-major -> sbuf [p=128, kt=4, d=64]; c = kt*128+p
                src = bass.AP(tensor=wdr, offset=0,
                              ap=[[HD, 128], [128 * HD, CT], [1, HD]])
                nc.sync.dma_start(out=wsb[:], in_=src)
            wo_sb = consts.tile([HD, C], bf16)
            nc.sync.dma_start(out=wo_sb[:], in_=wo_t[:, :])
            bqk_sb = consts.tile([128, 2], f32)
            nc.sync.dma_start(out=bqk_sb[:], in_=bqk2[:, :])
            bv_bc = consts.tile([128, HD], f32)
            nc.sync.dma_start(
                out=bv_bc[:],
                in_=bass.AP(tensor=bv_p, offset=0, ap=[[0, 128], [1, HD]]),
            )
            g_sb = consts.tile([128, CT], f32)
            b_sb = consts.tile([128, CT], f32)
            nc.sync.dma_start(out=g_sb[:], in_=bass.AP(tensor=gam, offset=0, ap=[[1, 128], [128, CT]]))
            nc.sync.dma_start(out=b_sb[:], in_=bass.AP(tensor=bet, offset=0, ap=[[1, 128], [128, CT]]))
            ind16_sb = consts.tile([128, 8], f32)
            nc.sync.dma_start(out=ind16_sb[:], in_=ind16[:, :])
            exp8_sb = consts.tile([8, 128], f32)
            nc.sync.dma_start(out=exp8_sb[:], in_=exp8[:, :])
            ones_sb = consts.tile([1, HD], f32)
            nc.vector.memset(ones_sb[:], 1.0)
            eps_sb = consts.tile([8, 1], f32)
            nc.vector.memset(eps_sb[:], EPS)

            # persistent activation tensors
            qT = consts.tile([128, N], bf16, tag="qT")
            kT = consts.tile([128, N], bf16, tag="kT")
            v_ext = [consts.tile([128, NJT, HD + 1], bf16, tag=f"vext{b}", name=f"vext{b}") for b in range(B)]
            for b in range(B):
                nc.vector.memset(v_ext[b][:, :, HD:HD + 1], 1.0)

            h_tiles = [[None] * CT for _ in range(B)]
            x_tiles = [[None] * CT for _ in range(B)]

            # ---------- GroupNorm ----------
            with tc.tile_pool(name="gnps", bufs=2, space="PSUM") as gnps, \
                 tc.tile_pool(name="gnps2", bufs=2, space="PSUM") as gnps2:
                for b in range(B):
                    psum_g = gnps.tile([8, 3 * CT], f32, tag="psg")
                    for ct in range(CT):
                        xt = xpool.tile([128, N], bf16, tag="xt")
                        x_tiles[b][ct] = xt
                        nc.sync.dma_start(out=xt[:], in_=xbf[b, ct * 128:(ct + 1) * 128, :])
                        stats = gnsb.tile([128, 8, 6], f32, tag="stats")
                        for s in range(8):
                            nc.vector.bn_stats(out=stats[:, s, :], in_=xt[:, s * 512:(s + 1) * 512])
                        mv = gnsb.tile([128, 2], f32, tag="mv")
                        nc.vector.bn_aggr(out=mv[:], in_=stats[:])
                        exs = gnsb.tile([128, 3], f32, tag="exs")
                        nc.vector.tensor_copy(out=exs[:, 0:2], in_=mv[:])
                        nc.vector.tensor_tensor(out=exs[:, 2:3], in0=mv[:, 0:1], in1=mv[:, 0:1], op=mult)
                        nc.tensor.matmul(psum_g[:, 3 * ct:3 * ct + 3], ind16_sb[:], exs[:],
                                         start=True, stop=True)
                    gst = gnsb.tile([8, 3 * CT], f32, tag="gst")
                    nc.vector.tensor_copy(out=gst[:], in_=psum_g[:])
                    for ct in range(CT):
                        c0 = gst[:, 3 * ct + 0:3 * ct + 1]
                        c1 = gst[:, 3 * ct + 1:3 * ct + 2]
                        c2 = gst[:, 3 * ct + 2:3 * ct + 3]
                        varg = small.tile([8, 1], f32, tag="varg")
                        sq0 = small.tile([8, 1], f32, tag="sq0")
                        nc.vector.tensor_tensor(out=varg[:], in0=c1, in1=c2, op=add)
                        nc.vector.tensor_tensor(out=sq0[:], in0=c0, in1=c0, op=mult)
                        nc.vector.tensor_tensor(out=varg[:], in0=varg[:], in1=sq0[:], op=subtract)
                        # rstd = exp(-0.5 * ln(var + eps)); Ln+Exp share one ACT table set
                        lnv = small.tile([8, 1], f32, tag="lnv")
                        nc.scalar.activation(out=lnv[:], in_=varg[:], func=AF.Ln, bias=eps_sb[:])
                        gv = small.tile([8, 2], f32, tag="gv")
                        nc.scalar.activation(out=gv[:, 1:2], in_=lnv[:], func=AF.Exp, scale=-0.5)
                        nc.vector.tensor_copy(out=gv[:, 0:1], in_=c0)
                        psum_e = gnps2.tile([128, 2], f32, tag="pse")
                        nc.tensor.matmul(psum_e[:], exp8_sb[:], gv[:], start=True, stop=True)
                        scl = small.tile([128, 1], f32, tag="scl")
                        tmp = small.tile([128, 1], f32, tag="tmp")
                        bia = small.tile([128, 1], f32, tag="bia")
                        nc.vector.tensor_tensor(out=scl[:], in0=psum_e[:, 1:2], in1=g_sb[:, ct:ct + 1], op=mult)
                        nc.vector.tensor_tensor(out=tmp[:], in0=psum_e[:, 0:1], in1=scl[:], op=mult)
                        nc.vector.tensor_tensor(out=bia[:], in0=b_sb[:, ct:ct + 1], in1=tmp[:], op=subtract)
                        ht = hpool.tile([128, N], bf16, tag="ht")
                        h_tiles[b][ct] = ht
                        nc.vector.tensor_scalar(out=ht[:], in0=x_tiles[b][ct][:],
                                                scalar1=scl[:], scalar2=bia[:],
                                                op0=mult, op1=add)

            # ---------- V (j-major layout), then Q/K (batch-pair col-tiled) ----------
            with tc.tile_pool(name="vps", bufs=2, space="PSUM") as vps, \
                 tc.tile_pool(name="qkps", bufs=2, space="PSUM") as qkps:
                def emit_v(b):
                    for jt in range(NJT):
                        pv = vps.tile([128, HD], f32, tag="pv")
                        for kt in range(CT):
                            nc.tensor.matmul(pv[:],
                                             h_tiles[b][kt][:, jt * 128:(jt + 1) * 128],
                                             wv_sb[:, kt, :],
                                             start=(kt == 0), stop=(kt == CT - 1))
                        nc.vector.tensor_tensor(out=v_ext[b][:, jt, 0:HD], in0=pv[:], in1=bv_bc[:], op=add)

                emit_v(0)
                # q then k, both batches col-tiled into one psum tile
                for (wsb, dest, bcol) in ((wq_sb, qT, 0), (wk_sb, kT, 1)):
                    for ic in range(NIC):
                        pq = qkps.tile([128, 512], f32, tag="pq")
                        for kt in range(CT):
                            nc.tensor.matmul(pq[0:64, :], wsb[:, kt, :],
                                             h_tiles[0][kt][:, ic * 512:(ic + 1) * 512],
                                             start=(kt == 0), stop=(kt == CT - 1),
                                             tile_position=(0, 0))
                            nc.tensor.matmul(pq[64:128, :], wsb[:, kt, :],
                                             h_tiles[1][kt][:, ic * 512:(ic + 1) * 512],
                                             start=(kt == 0), stop=(kt == CT - 1),
                                             tile_position=(0, 64), skip_group_check=True)
                        nc.vector.tensor_scalar(out=dest[:, ic * 512:(ic + 1) * 512],
                                                in0=pq[:],
                                                scalar1=bqk_sb[:, bcol:bcol + 1], scalar2=None,
                                                op0=add)
                emit_v(1)

            # ---------- attention + output projection ----------
            with tc.tile_pool(name="stps", bufs=2, space="PSUM") as stps, \
                 tc.tile_pool(name="ops", bufs=2, space="PSUM") as ops, \
                 tc.tile_pool(name="wops", bufs=2, space="PSUM") as wops:
                for ic in range(NIC):
                    o_ps = [ops.tile([HD + 1, 512], f32, tag="ops", name=f"ops{ic}_{bb}") for bb in range(B)]
                    for jt in range(NJT):
                        st = stps.tile([128, 1024], f32, tag="st")
                        nc.tensor.matmul(st[:, 0:512],
                                         kT[0:64, jt * 128:(jt + 1) * 128],
                                         qT[0:64, ic * 512:(ic + 1) * 512],
                                         start=True, stop=True, tile_position=(0, 0))
                        nc.tensor.matmul(st[:, 512:1024],
                                         kT[64:128, jt * 128:(jt + 1) * 128],
                                         qT[64:128, ic * 512:(ic + 1) * 512],
                                         start=True, stop=True, tile_position=(64, 0))
                        pt = ptpool.tile([128, 1024], bf16, tag="pt")
                        nc.scalar.activation(out=pt[:], in_=st[:], func=AF.Exp, scale=SM_SCALE)
                        for b in range(B):
                            nc.tensor.matmul(o_ps[b][:],
                                             v_ext[b][:, jt, :],
                                             pt[:, b * 512:(b + 1) * 512],
                                             start=(jt == 0), stop=(jt == NJT - 1))
                    for b in range(B):
                        rc = small.tile([1, 512], f32, tag="rc")
                        nc.vector.reciprocal(out=rc[:], in_=o_ps[b][HD:HD + 1, :])
                        bc_ps = wops.tile([64, 512], f32, tag="w")
                        nc.tensor.matmul(bc_ps[:], ones_sb[:], rc[:], start=True, stop=True)
                        bc_sb = small.tile([64, 512], bf16, tag="bc")
                        nc.vector.tensor_copy(out=bc_sb[:], in_=bc_ps[:])
                        oN = onpool.tile([64, 512], bf16, tag="oN")
                        nc.vector.tensor_tensor(out=oN[:], in0=o_ps[b][0:HD, :], in1=bc_sb[:], op=mult)
                        for mt in range(CT):
                            wp = wops.tile([128, 512], f32, tag="w")
                            nc.tensor.matmul(wp[:], wo_sb[:, mt * 128:(mt + 1) * 128], oN[:],
                                             start=True, stop=True)
                            ob = outp.tile([128, 512], f32, tag="ob")
                            nc.vector.tensor_copy(out=ob[:], in_=wp[:])
                            nc.sync.dma_start(
                                out=out[b, mt * 128:(mt + 1) * 128, ic * 512:(ic + 1) * 512],
                                in_=ob[:])
    _split_excess_waits(nc, mybir)
    return nc


def _prep_in_maps(inputs):
    from concourse import mybir

    np_bf16 = mybir.dt.np(mybir.dt.bfloat16)
    x = np.asarray(inputs["x"], np.float32)
    gamma = np.asarray(inputs["gamma"], np.float32)
    beta = np.asarray(inputs["beta"], np.float32)
    Wq = np.asarray(inputs["Wq"], np.float32)
    bq = np.asarray(inputs["bq"], np.float32)
    Wk = np.asarray(inputs["Wk"], np.float32)
    bk = np.asarray(inputs["bk"], np.float32)
    Wv = np.asarray(inputs["Wv"], np.float32)
    bv = np.asarray(inputs["bv"], np.float32)
    Wo = np.asarray(inputs["Wo"], np.float32)

    xbf = np.ascontiguousarray(x.reshape(B, C, N)).astype(np_bf16)
    ind16 = np.zeros((128, 8), np.float32)
    for p in range(128):
        ind16[p, p // 16] = 1.0 / 16.0
    exp8 = np.zeros((8, 128), np.float32)
    for p in range(128):
        exp8[p // 16, p] = 1.0
    gam2 = np.ascontiguousarray(gamma.reshape(C, 1))
    bet2 = np.ascontiguousarray(beta.reshape(C, 1))

    in_maps = []
    for c in range(NUM_HEADS):
        sl = slice(c * HD, (c + 1) * HD)
        bqk2 = np.stack([np.tile(bq[sl], 2), np.tile(bk[sl], 2)], axis=1)
        in_maps.append({
            "xbf": xbf,
            "wq_t": np.ascontiguousarray(Wq[sl, :].T).astype(np_bf16),
            "wk_t": np.ascontiguousarray(Wk[sl, :].T).astype(np_bf16),
            "wv_t": np.ascontiguousarray(Wv[sl, :].T).astype(np_bf16),
            "wo_t": np.ascontiguousarray(Wo[:, sl].T).astype(np_bf16),
            "bqk2": np.ascontiguousarray(bqk2, dtype=np.float32),
            "bv": np.ascontiguousarray(bv[sl]),
            "gam": gam2,
            "bet": bet2,
            "ind16": ind16,
            "exp8": exp8,
        })
    return in_maps


def kernel(**inputs):
    from concourse.bass_utils import run_bass_kernel_spmd

    if "nc" not in _CACHE:
        _CACHE["nc"] = build_program()
    nc = _CACHE["nc"]
    in_maps = _prep_in_maps(inputs)
    res = run_bass_kernel_spmd(nc, in_maps, core_ids=list(range(NUM_HEADS)))
    x = np.asarray(inputs["x"], np.float32)
    bo = np.asarray(inputs["bo"], np.float32)
    acc = np.zeros((B, C, N), np.float32)
    for c in range(NUM_HEADS):
        acc += res.results[c]["out"]
    acc += bo[None, :, None]
    return (x + acc.reshape(B, C, H, W)).astype(np.float32)
